# revision 33
# baseline (speedup 1.0000x reference)
"""Multi-head attention (B=2, S=2048, D=1024, H=16, causal) on 8 TRN2 NeuronCores.

Sharding: core c -> (batch b = c//4, head-group hg = c%4). Each core:
  - projects its batch's query/key/value against a 256-row slice of Wq/Wk/Wv
    (4 heads of 64 dims),
  - runs causal attention for those 4 heads (scores computed transposed,
    exp on ACT with fused 1/8 scale, row-sums via a ones-column in V),
  - multiplies by the matching 256-column slice of Wo -> partial [2048, 1024].
Host sums the 4 partials per batch (the tensor-parallel all-reduce) and stacks.

Schedule: a single software-pipelined stream. Projection and out-projection
matmul groups are interleaved as "filler" units into the attention chunk
stream so the PE never stalls on the ACT exp chain (which would let the HAM
clock gate re-throttle the PE from 2.4 to 1.2 GHz). Score chunks are computed
in pairs sharing one 2-bank PSUM tile so each ACT exp instruction covers two
chunks (halves the per-instruction ACT overhead). Softmax normalization is
two-phase: a DVE fast-reciprocal chain, then ones-matmul broadcast + scale
deferred into the filler queue so the PE FIFO never head-of-line blocks on
it. Dummy full-array matmuls at kernel start keep the HAM activity monitor
busy while the first DMAs land, so real matmuls start at the warm clock.
Input DMAs issue on one queue in strict consumer order (a queue's transfers
complete in issue order, so first-needed tensors land first); blocks 2-3 are
paced by the xin pool's slot semaphores.

Precision: all matmul operands are bf16 (fp32 accumulate in PSUM); exp output,
causal mask and normalization scales are bf16; softmax row-sums/reciprocals
stay fp32 (reciprocal_approx_fast, ~18 bits). Measured end-to-end error vs
the fp32 reference ~4.4e-3 (gate: 2e-2).

Measured: 282us (f32r baseline) -> ~179us on TRN2.
"""

import sys

for _p in ("/opt/trn_rl_repo", "/root/.axon_site/_ro/trn_rl_repo"):
    if _p not in sys.path:
        sys.path.append(_p)

from collections import deque

import numpy as np
import ml_dtypes

import concourse.bacc as bacc
import concourse.tile as tile
import concourse.mybir as mybir
from concourse.bass import MemorySpace
from concourse.bass_utils import run_bass_kernel_spmd

f32 = mybir.dt.float32
bf16 = mybir.dt.bfloat16
Exp = mybir.ActivationFunctionType.Exp

B, S, D, H = 2, 2048, 1024, 16
HD = 64            # head dim
NH = 4             # heads per core
DO = NH * HD       # 256 projection out-dims per core
NCORES = 8
KI = D // 128      # 8 contraction chunks for the projections
QT = 512           # query tile
NQT = S // QT      # 4
KT = 128           # key chunk (contraction tile for PV)
NT = QT // KT      # 4 key chunks per token block

_cache: dict = {}

# ablation switches for perf experiments (leave defaults for production)
_opts = {"leadp": 1, "mask_engine": "pool", "pair_exp": True}


def _build(repeat: int = 1):
    nc = bacc.Bacc("TRN2", target_bir_lowering=False, debug=False,
                   num_devices=NCORES)

    xqT_d = nc.dram_tensor("xqT", [D, S], bf16, kind="ExternalInput").ap()
    xkT_d = nc.dram_tensor("xkT", [D, S], bf16, kind="ExternalInput").ap()
    xvT_d = nc.dram_tensor("xvT", [D, S], bf16, kind="ExternalInput").ap()
    wqT_d = nc.dram_tensor("wqT", [D, DO], bf16, kind="ExternalInput").ap()
    wkT_d = nc.dram_tensor("wkT", [D, DO], bf16, kind="ExternalInput").ap()
    wvT_d = nc.dram_tensor("wvT", [D, DO], bf16, kind="ExternalInput").ap()
    woT_d = nc.dram_tensor("woT", [DO, D], bf16, kind="ExternalInput").ap()
    cmask_d = nc.dram_tensor("cmask", [128, KT], bf16, kind="ExternalInput").ap()
    out_d = nc.dram_tensor("out", [S, D], bf16, kind="ExternalOutput").ap()

    with tile.TileContext(nc) as tc:
        with (
            tc.tile_pool(name="wpool", bufs=1) as wpool,
            tc.tile_pool(name="cpool", bufs=1) as cpool,
            tc.tile_pool(name="persist", bufs=1) as persist,
            tc.tile_pool(name="xin", bufs=6) as xin,
            tc.tile_pool(name="ptp", bufs=4) as ptp,
            tc.tile_pool(name="small", bufs=2) as small,
            tc.tile_pool(name="obuf", bufs=3) as obuf,
            tc.tile_pool(name="psS", bufs=2, space=MemorySpace.PSUM) as psS,
            tc.tile_pool(name="psA", bufs=2, space=MemorySpace.PSUM) as psA,
            tc.tile_pool(name="psO", bufs=2, space=MemorySpace.PSUM) as psO,
        ):
            pools = (nc, wpool, cpool, persist, xin, ptp, small, obuf,
                     psS, psA, psO, xqT_d, xkT_d, xvT_d, wqT_d, wkT_d,
                     wvT_d, woT_d, cmask_d, out_d)
            if repeat > 1:
                with tc.For_i(0, repeat):
                    _emit(*pools)
            else:
                _emit(*pools)

    nc.compile()
    return nc


def _emit(nc, wpool, cpool, persist, xin, ptp, small, obuf, psS, psA, psO,
          xqT_d, xkT_d, xvT_d, wqT_d, wkT_d, wvT_d, woT_d, cmask_d, out_d):
    # ---- DMA issue order = consumer order: the sync queue's counting
    # semaphore makes every consumer wait for ALL earlier-issued DMAs,
    # so weights/inputs are issued exactly in first-use order.
    wq_sb = wpool.tile([128, KI, DO], bf16, tag="wq")
    wk_sb = wpool.tile([128, KI, DO], bf16, tag="wk")
    wv_sb = wpool.tile([128, KI, DO], bf16, tag="wv")
    wo_sb = wpool.tile([128, DO // 128, D], bf16, tag="wo")
    tri_sb = cpool.tile([128, KT], bf16, tag="tri")

    # All input DMAs go on ONE queue (sync) in strict consumer order: a hw
    # queue's transfers complete in issue order at full aggregate bandwidth,
    # so the first-needed tensors land first. Blocks 2-3 rotate through the
    # xin pool (bufs=6): their dma_start waits on the slot semaphore, which
    # paces the prefetch ~2 blocks ahead without racing early transfers.
    # The scalar queue carries no DMAs so exp issue is never delayed.
    xq, xk, xv = [None] * NQT, [None] * NQT, [None] * NQT

    def dma_in(sb, dram, t=None):
        src = dram if t is None else dram[:, t * QT:(t + 1) * QT]
        nc.sync.dma_start(sb[:], src.rearrange("(k p) n -> p k n", p=128))

    def load_block(t):
        xq[t] = xin.tile([128, KI, QT], bf16, tag="xin", name=f"xq{t}")
        dma_in(xq[t], xqT_d, t)
        xk[t] = xin.tile([128, KI, QT], bf16, tag="xin", name=f"xk{t}")
        dma_in(xk[t], xkT_d, t)
        xv[t] = xin.tile([128, KI, QT], bf16, tag="xin", name=f"xv{t}")
        dma_in(xv[t], xvT_d, t)

    dma_in(wq_sb, wqT_d)
    xq[0] = xin.tile([128, KI, QT], bf16, tag="xin", name="xq0")
    dma_in(xq[0], xqT_d, 0)
    dma_in(wk_sb, wkT_d)
    xk[0] = xin.tile([128, KI, QT], bf16, tag="xin", name="xk0")
    dma_in(xk[0], xkT_d, 0)
    dma_in(wv_sb, wvT_d)
    xv[0] = xin.tile([128, KI, QT], bf16, tag="xin", name="xv0")
    dma_in(xv[0], xvT_d, 0)
    nc.sync.dma_start(tri_sb[:], cmask_d)
    load_block(1)
    dma_in(wo_sb, woT_d)
    load_block(2)
    load_block(3)

    # ---- per-block persistent intermediates ----
    # qT/kT/oT blocks: [256, QT] as [128 parts, 2 chunks, QT]
    #   head j lives in chunk j//2, partitions (j%2)*64 ..+64
    qTt = [persist.tile([128, 2, QT], bf16, tag=f"qT{t}", name=f"qT{t}")
           for t in range(NQT)]
    kTt = [persist.tile([128, 2, QT], bf16, tag=f"kT{t}", name=f"kT{t}")
           for t in range(NQT)]
    oTt = [persist.tile([128, 2, QT], bf16, tag=f"oT{t}", name=f"oT{t}")
           for t in range(NQT)]
    # v blocks, natural layout + ones column: [tokk part, ktc, head, 65]
    vt = [persist.tile([128, NT, NH, HD + 1], bf16, tag=f"v{t}", name=f"v{t}")
          for t in range(NQT)]

    vones = cpool.tile([128, NT * NH], bf16, tag="vones")
    nc.vector.memset(vones[:], 1.0)
    ones_bc = cpool.tile([128, HD], bf16, tag="ones_bc")
    nc.vector.memset(ones_bc[:], 1.0)
    wpad = cpool.tile([128, QT], bf16, tag="wpad")
    nc.vector.memset(wpad[:], 0.0)
    for t in range(NQT):
        nc.vector.tensor_copy(
            vt[t][:, :, :, HD], vones[:].rearrange("p (a b) -> p a b", a=NT))

    def warmup(n):
        # dummy matmuls with no DMA dependency: keep the PE's HAM activity
        # window busy while input DMAs land, so real matmuls start at the
        # warm 2.4 GHz clock instead of cold 1.2 GHz
        for _ in range(n):
            psD = psO.tile([HD + 1, QT], f32, tag="pso", name="psD")
            nc.tensor.matmul(psD[0:HD, :], ones_bc[:], wpad[:],
                             start=True, stop=True)

    # ---- emission units (each ~0.4-1.7us of PE work) ----
    def unit_proj_qk(t, d, which):
        w_sb, x_sb, dst = ((wq_sb, xq[t], qTt[t]) if which == "q"
                           else (wk_sb, xk[t], kTt[t]))

        def emit():
            ps = psA.tile([128, QT], f32, tag="ps", name="ps")
            for ki in range(KI):
                nc.tensor.matmul(
                    ps[:], w_sb[:, ki, d * 128:(d + 1) * 128],
                    x_sb[:, ki, :], start=(ki == 0), stop=(ki == KI - 1))
            nc.vector.tensor_copy(dst[:, d, :], ps[:])
        return emit

    def unit_proj_v(t, tt):
        def emit():
            psv = psA.tile([128, QT], f32, tag="ps")
            for ki in range(KI):
                nc.tensor.matmul(
                    psv[:, 0:DO], xv[t][:, ki, tt * KT:(tt + 1) * KT],
                    wv_sb[:, ki, :], start=(ki == 0), stop=(ki == KI - 1))
            nc.vector.tensor_copy(
                vt[t][:, tt, :, 0:HD],
                psv[:, 0:DO].rearrange("p (h e) -> p h e", h=NH))
        return emit

    def unit_oproj(t, mtt, n, alt_copy=False):
        def emit():
            ps = psA.tile([128, QT], f32, tag="ps")
            for kc in range(DO // 128):
                nc.tensor.matmul(
                    ps[:], oTt[t][:, kc, mtt * KT:(mtt + 1) * KT],
                    wo_sb[:, kc, n * QT:(n + 1) * QT],
                    start=(kc == 0), stop=(kc == DO // 128 - 1))
            ob = obuf.tile([128, QT], bf16, tag="ob")
            if alt_copy:
                nc.scalar.copy(ob[:], ps[:])
            else:
                nc.vector.tensor_copy(ob[:], ps[:])
            mt = t * NT + mtt
            nc.sync.dma_start(
                out_d[mt * 128:(mt + 1) * 128, n * QT:(n + 1) * QT], ob[:])
        return emit

    def proj_units(t):
        return ([unit_proj_qk(t, d, w) for d in range(2) for w in ("q", "k")]
                + [unit_proj_v(t, tt) for tt in range(NT)])

    def oproj_units(t, alt=False):
        # alt: alternate the PSUM->SBUF copy between ACT and DVE so the
        # final (tail) out-projection is not paced by a single engine
        return [unit_oproj(t, mtt, n, alt_copy=alt and (mtt + n) % 2 == 0)
                for mtt in range(NT) for n in range(D // QT)]


    def attn_block(qt, fillers):
        """Attention for block qt with filler units interleaved into the PE
        stream (emitted just before each PV pair so a pending exp never
        head-of-line-blocks independent matmuls). Returns the deferred PE
        part of heads 2,3's normalization for the caller to emit later."""
        LEADP = _opts["leadp"]
        nkt = (qt + 1) * NT
        npairs = nkt // 2
        total_steps = NH * npairs
        fillers = deque(fillers)
        fill_acc, fill_step = 0.0, (len(fillers) / total_steps
                                    if total_steps else 0.0)
        # row-sum rows must sit at partition bases 0/32 (engine partition
        # offsets are 32-aligned and base 96 is illegal for PE):
        # heads 0,1 -> rsA rows 0,32; heads 2,3 -> rsB rows 0,32
        rsAB = [small.tile([64, QT], f32, tag="rsA", name="rsA"),
                small.tile([64, QT], f32, tag="rsB", name="rsB")]
        ouns = []

        def norm_dve(h):
            # DVE half of the normalization chain for heads 2h, 2h+1
            rcp = small.tile([64, QT], f32, tag="rcp")
            nc.vector.reciprocal_approx_fast(rcp[:], rsAB[h][:])
            rcpb = small.tile([64, QT], bf16, tag="rcpb", bufs=4)
            nc.vector.tensor_copy(rcpb[:], rcp[:])
            return rcpb

        def norm_pe_units(h, rcpb):
            # PE half (broadcast via ones-matmul) + final scale, as filler
            # units so they're emitted well after the DVE chain was queued
            def mk(j):
                def emit():
                    poff = (j % 2) * HD
                    psb = psA.tile([128, QT], f32, tag="ps")
                    nc.tensor.matmul(psb[0:HD, :],
                                     ones_bc[32 * (j % 2):32 * (j % 2) + 1, :],
                                     rcpb[32 * (j % 2):32 * (j % 2) + 1, :],
                                     start=True, stop=True)
                    bc = small.tile([HD, QT], bf16, tag="bc", bufs=4)
                    nc.vector.tensor_copy(bc[:], psb[0:HD, :])
                    nc.vector.tensor_mul(oTt[qt][poff:poff + HD, h, :],
                                         ouns[j][:], bc[:])
                return emit
            return [mk(2 * h), mk(2 * h + 1)]

        warmup(2)
        for j in range(NH):
            if j == 2:
                # heads 0,1 are complete: overlap their normalization with
                # heads 2,3's attention (PE part goes into the filler queue)
                fillers.extend(norm_pe_units(0, norm_dve(0)))
            poff = (j % 2) * HD
            d = j // 2
            qh = qTt[qt][poff:poff + HD, d, :]
            pso = psO.tile([HD + 1, QT], f32, tag="pso")
            pending = {}
            for step in range(npairs + LEADP):
                if step < npairs:
                    p = step
                    pss = psS.tile([128, 2, QT], f32, tag="pss")
                    pt = ptp.tile([128, 2, QT], bf16, tag="pt")
                    info = []
                    for i in (0, 1):
                        kt = 2 * p + i
                        r = kt - qt * NT
                        co = max(r, 0) * KT
                        w = QT - co
                        kh = kTt[kt // NT][poff:poff + HD, d,
                                           (kt % NT) * KT:(kt % NT + 1) * KT]
                        nc.tensor.matmul(pss[:, i, 0:w], kh, qh[:, co:QT],
                                         start=True, stop=True)
                        info.append((kt, co, w, r))
                    if _opts["pair_exp"]:
                        # one exp covers the pair, clipped to the wider
                        # chunk's width (pt beyond each chunk's own width is
                        # garbage and never read)
                        w0 = info[0][2]
                        nc.scalar.activation(pt[:, :, 0:w0], pss[:, :, 0:w0],
                                             Exp, scale=0.125)
                    else:
                        for i, (kt, co, w, r) in enumerate(info):
                            nc.scalar.activation(pt[:, i, 0:w],
                                                 pss[:, i, 0:w], Exp,
                                                 scale=0.125)
                    for i, (kt, co, w, r) in enumerate(info):
                        if r >= 0:
                            if _opts["mask_engine"] == "pool":
                                nc.gpsimd.tensor_mul(
                                    pt[:, i, 0:KT], pt[:, i, 0:KT], tri_sb[:])
                            else:
                                nc.vector.tensor_mul(
                                    pt[:, i, 0:KT], pt[:, i, 0:KT], tri_sb[:])
                    pending[p] = (pt, info)
                # fillers go in front of the PV pair: PV waits on exp, the
                # filler must not sit behind it in the PE FIFO
                fill_acc += fill_step
                while fill_acc >= 1.0 and fillers:
                    fillers.popleft()()
                    fill_acc -= 1.0
                if step >= LEADP:
                    pt, info = pending.pop(step - LEADP)
                    for i, (kt, co, w, r) in enumerate(info):
                        nc.tensor.matmul(
                            pso[:, co:QT], vt[kt // NT][:, kt % NT, j, :],
                            pt[:, i, 0:w],
                            start=(kt == 0), stop=(kt == nkt - 1))
            # free the PSUM accumulator; split row-sum row into the batch tile
            oun = small.tile([HD, QT], bf16, tag="oun", bufs=5)
            nc.vector.tensor_copy(oun[:], pso[0:HD, :])
            nc.vector.tensor_copy(rsAB[j // 2][32 * (j % 2):32 * (j % 2) + 1, :],
                                  pso[HD:HD + 1, :])
            ouns.append(oun)
        while fillers:
            fillers.popleft()()
        return norm_pe_units(1, norm_dve(1))

    # ---- software-pipelined schedule ----
    # carry (the deferred norm-PE units) is placed mid-filler-stream so it
    # never reaches the PE before its DVE reciprocal chain has drained
    warmup(8)
    for u in proj_units(0):
        warmup(2)
        u()
    c = attn_block(0, proj_units(1))
    c = attn_block(1, proj_units(2) + c)
    c = attn_block(2, proj_units(3) + c + oproj_units(0))
    c = attn_block(3, oproj_units(1) + c + oproj_units(2))
    warmup(4)
    for u in c:
        u()
    for u in oproj_units(3, alt=True):
        u()


def _bf16(x: np.ndarray) -> np.ndarray:
    return np.ascontiguousarray(np.asarray(x, np.float32)).astype(
        ml_dtypes.bfloat16)


def _mask_tiles() -> np.ndarray:
    i = np.arange(128)[:, None]
    j = np.arange(KT)[None, :]
    return (j >= i).astype(np.float32)


def make_in_maps(query, key, value, Wq, Wk, Wv, Wo):
    query = np.asarray(query, np.float32)
    key = np.asarray(key, np.float32)
    value = np.asarray(value, np.float32)
    Wq = np.asarray(Wq, np.float32)
    Wk = np.asarray(Wk, np.float32)
    Wv = np.asarray(Wv, np.float32)
    Wo = np.asarray(Wo, np.float32)
    cm = _bf16(_mask_tiles())
    in_maps = []
    for c in range(NCORES):
        b, hg = divmod(c, NCORES // B)
        sl = slice(hg * DO, (hg + 1) * DO)
        in_maps.append({
            "xqT": _bf16(query[b].T),
            "xkT": _bf16(key[b].T),
            "xvT": _bf16(value[b].T),
            "wqT": _bf16(Wq[sl].T),
            "wkT": _bf16(Wk[sl].T),
            "wvT": _bf16(Wv[sl].T),
            "woT": _bf16(Wo[:, sl].T),
            "cmask": cm,
        })
    return in_maps


def kernel(query, key, value, freqs_complex_form, mask, Wq, Wk, Wv, Wo):
    if "nc" not in _cache:
        _cache["nc"] = _build()
    nc = _cache["nc"]
    in_maps = make_in_maps(query, key, value, Wq, Wk, Wv, Wo)
    res = run_bass_kernel_spmd(nc, in_maps, list(range(NCORES)))
    npg = NCORES // B
    parts = [np.asarray(res.results[c]["out"], dtype=np.float32)
             for c in range(NCORES)]
    return np.stack(
        [np.sum(parts[b * npg:(b + 1) * npg], axis=0) for b in range(B)]
    ).astype(np.float32)


# revision 35
# speedup vs baseline: 1.0257x; 1.0257x over previous
"""Multi-head attention (B=2, S=2048, D=1024, H=16, causal) on 8 TRN2 NeuronCores.

Sharding: core c -> (batch b = c//4, head-group hg = c%4). Each core:
  - projects its batch's query/key/value against a 256-row slice of Wq/Wk/Wv
    (4 heads of 64 dims),
  - runs causal attention for those 4 heads (scores computed transposed,
    exp on ACT with fused 1/8 scale, row-sums via a ones-column in V),
  - multiplies by the matching 256-column slice of Wo -> partial [2048, 1024].
Host sums the 4 partials per batch (the tensor-parallel all-reduce) and stacks.

Schedule: a single software-pipelined stream. Projection and out-projection
matmul groups are interleaved as "filler" units into the attention chunk
stream so the PE never stalls on the ACT exp chain (which would let the HAM
clock gate re-throttle the PE from 2.4 to 1.2 GHz). Score chunks are computed
in pairs sharing one 2-bank PSUM tile so each ACT exp instruction covers two
chunks (halves the per-instruction ACT overhead). Softmax normalization is
two-phase: a DVE fast-reciprocal chain, then ones-matmul broadcast + scale
deferred into the filler queue so the PE FIFO never head-of-line blocks on
it. Dummy full-array matmuls at kernel start keep the HAM activity monitor
busy while the first DMAs land, so real matmuls start at the warm clock.
Input DMAs issue on one queue in strict consumer order (a queue's transfers
complete in issue order, so first-needed tensors land first); blocks 2-3 are
paced by the xin pool's slot semaphores.

Precision: all matmul operands are bf16 (fp32 accumulate in PSUM); exp output,
causal mask and normalization scales are bf16; softmax row-sums/reciprocals
stay fp32 (reciprocal_approx_fast, ~18 bits). Measured end-to-end error vs
the fp32 reference ~4.4e-3 (gate: 2e-2).

Measured: 282us (f32r baseline) -> ~179us on TRN2.
"""

import sys

for _p in ("/opt/trn_rl_repo", "/root/.axon_site/_ro/trn_rl_repo"):
    if _p not in sys.path:
        sys.path.append(_p)

from collections import deque

import numpy as np
import ml_dtypes

import concourse.bacc as bacc
import concourse.tile as tile
import concourse.mybir as mybir
from concourse.bass import MemorySpace
from concourse.bass_utils import run_bass_kernel_spmd

f32 = mybir.dt.float32
bf16 = mybir.dt.bfloat16
Exp = mybir.ActivationFunctionType.Exp

B, S, D, H = 2, 2048, 1024, 16
HD = 64            # head dim
NH = 4             # heads per core
DO = NH * HD       # 256 projection out-dims per core
NCORES = 8
KI = D // 128      # 8 contraction chunks for the projections
QT = 512           # query tile
NQT = S // QT      # 4
KT = 128           # key chunk (contraction tile for PV)
NT = QT // KT      # 4 key chunks per token block

_cache: dict = {}

# ablation switches for perf experiments (leave defaults for production)
_opts = {"leadp": 1, "mask_engine": "pool", "pair_exp": True}


def _build(repeat: int = 1):
    nc = bacc.Bacc("TRN2", target_bir_lowering=False, debug=False,
                   num_devices=NCORES)

    xqT_d = nc.dram_tensor("xqT", [D, S], bf16, kind="ExternalInput").ap()
    xkT_d = nc.dram_tensor("xkT", [D, S], bf16, kind="ExternalInput").ap()
    xvT_d = nc.dram_tensor("xvT", [D, S], bf16, kind="ExternalInput").ap()
    wqT_d = nc.dram_tensor("wqT", [D, DO], bf16, kind="ExternalInput").ap()
    wkT_d = nc.dram_tensor("wkT", [D, DO], bf16, kind="ExternalInput").ap()
    wvT_d = nc.dram_tensor("wvT", [D, DO], bf16, kind="ExternalInput").ap()
    woT_d = nc.dram_tensor("woT", [DO, D], bf16, kind="ExternalInput").ap()
    cmask_d = nc.dram_tensor("cmask", [128, KT], bf16, kind="ExternalInput").ap()
    out_d = nc.dram_tensor("out", [S, D], bf16, kind="ExternalOutput").ap()

    with tile.TileContext(nc) as tc:
        with (
            tc.tile_pool(name="wpool", bufs=1) as wpool,
            tc.tile_pool(name="cpool", bufs=1) as cpool,
            tc.tile_pool(name="persist", bufs=1) as persist,
            tc.tile_pool(name="xin", bufs=6) as xin,
            tc.tile_pool(name="ptp", bufs=4) as ptp,
            tc.tile_pool(name="small", bufs=2) as small,
            tc.tile_pool(name="obuf", bufs=3) as obuf,
            tc.tile_pool(name="psS", bufs=2, space=MemorySpace.PSUM) as psS,
            tc.tile_pool(name="psA", bufs=2, space=MemorySpace.PSUM) as psA,
            tc.tile_pool(name="psO", bufs=2, space=MemorySpace.PSUM) as psO,
        ):
            pools = (nc, wpool, cpool, persist, xin, ptp, small, obuf,
                     psS, psA, psO, xqT_d, xkT_d, xvT_d, wqT_d, wkT_d,
                     wvT_d, woT_d, cmask_d, out_d)
            if repeat > 1:
                with tc.For_i(0, repeat):
                    _emit(*pools)
            else:
                _emit(*pools)

    nc.compile()
    return nc


def _emit(nc, wpool, cpool, persist, xin, ptp, small, obuf, psS, psA, psO,
          xqT_d, xkT_d, xvT_d, wqT_d, wkT_d, wvT_d, woT_d, cmask_d, out_d):
    # ---- DMA issue order = consumer order: the sync queue's counting
    # semaphore makes every consumer wait for ALL earlier-issued DMAs,
    # so weights/inputs are issued exactly in first-use order.
    wq_sb = wpool.tile([128, KI, DO], bf16, tag="wq")
    wk_sb = wpool.tile([128, KI, DO], bf16, tag="wk")
    wv_sb = wpool.tile([128, KI, DO], bf16, tag="wv")
    wo_sb = wpool.tile([128, DO // 128, D], bf16, tag="wo")
    tri_sb = cpool.tile([128, KT], bf16, tag="tri")

    # All input DMAs go on ONE queue (sync) in strict consumer order: a hw
    # queue's transfers complete in issue order at full aggregate bandwidth,
    # so the first-needed tensors land first. Blocks 2-3 rotate through the
    # xin pool (bufs=6): their dma_start waits on the slot semaphore, which
    # paces the prefetch ~2 blocks ahead without racing early transfers.
    # The scalar queue carries no DMAs so exp issue is never delayed.
    xq, xk, xv = [None] * NQT, [None] * NQT, [None] * NQT

    def dma_in(sb, dram, t=None):
        src = dram if t is None else dram[:, t * QT:(t + 1) * QT]
        nc.sync.dma_start(sb[:], src.rearrange("(k p) n -> p k n", p=128))

    def load_block(t):
        xq[t] = xin.tile([128, KI, QT], bf16, tag="xin", name=f"xq{t}")
        dma_in(xq[t], xqT_d, t)
        xk[t] = xin.tile([128, KI, QT], bf16, tag="xin", name=f"xk{t}")
        dma_in(xk[t], xkT_d, t)
        xv[t] = xin.tile([128, KI, QT], bf16, tag="xin", name=f"xv{t}")
        dma_in(xv[t], xvT_d, t)

    dma_in(wq_sb, wqT_d)
    xq[0] = xin.tile([128, KI, QT], bf16, tag="xin", name="xq0")
    dma_in(xq[0], xqT_d, 0)
    dma_in(wk_sb, wkT_d)
    xk[0] = xin.tile([128, KI, QT], bf16, tag="xin", name="xk0")
    dma_in(xk[0], xkT_d, 0)
    dma_in(wv_sb, wvT_d)
    xv[0] = xin.tile([128, KI, QT], bf16, tag="xin", name="xv0")
    dma_in(xv[0], xvT_d, 0)
    nc.sync.dma_start(tri_sb[:], cmask_d)
    load_block(1)
    dma_in(wo_sb, woT_d)
    load_block(2)
    load_block(3)

    # ---- per-block persistent intermediates ----
    # qT/kT/oT blocks: [256, QT] as [128 parts, 2 chunks, QT]
    #   head j lives in chunk j//2, partitions (j%2)*64 ..+64
    qTt = [persist.tile([128, 2, QT], bf16, tag=f"qT{t}", name=f"qT{t}")
           for t in range(NQT)]
    kTt = [persist.tile([128, 2, QT], bf16, tag=f"kT{t}", name=f"kT{t}")
           for t in range(NQT)]
    oTt = [persist.tile([128, 2, QT], bf16, tag=f"oT{t}", name=f"oT{t}")
           for t in range(NQT)]
    # v blocks, natural layout + ones column: [tokk part, ktc, head, 65]
    vt = [persist.tile([128, NT, NH, HD + 1], bf16, tag=f"v{t}", name=f"v{t}")
          for t in range(NQT)]

    vones = cpool.tile([128, NT * NH], bf16, tag="vones")
    nc.vector.memset(vones[:], 1.0)
    ones_bc = cpool.tile([128, HD], bf16, tag="ones_bc")
    nc.vector.memset(ones_bc[:], 1.0)
    wpad = cpool.tile([128, QT], bf16, tag="wpad")
    nc.vector.memset(wpad[:], 0.0)
    for t in range(NQT):
        nc.vector.tensor_copy(
            vt[t][:, :, :, HD], vones[:].rearrange("p (a b) -> p a b", a=NT))

    def warmup(n):
        # dummy matmuls with no DMA dependency: keep the PE's HAM activity
        # window busy while input DMAs land, so real matmuls start at the
        # warm 2.4 GHz clock instead of cold 1.2 GHz
        for _ in range(n):
            psD = psO.tile([HD + 1, QT], f32, tag="pso", name="psD")
            nc.tensor.matmul(psD[0:HD, :], ones_bc[:], wpad[:],
                             start=True, stop=True)

    # ---- emission units (each ~0.4-1.7us of PE work) ----
    def unit_proj_qk(t, d, which):
        w_sb, x_sb, dst = ((wq_sb, xq[t], qTt[t]) if which == "q"
                           else (wk_sb, xk[t], kTt[t]))

        def emit():
            ps = psA.tile([128, QT], f32, tag="ps", name="ps")
            for ki in range(KI):
                nc.tensor.matmul(
                    ps[:], w_sb[:, ki, d * 128:(d + 1) * 128],
                    x_sb[:, ki, :], start=(ki == 0), stop=(ki == KI - 1))
            nc.vector.tensor_copy(dst[:, d, :], ps[:])
        return emit

    def unit_proj_v(t, tt):
        def emit():
            psv = psA.tile([128, QT], f32, tag="ps")
            for ki in range(KI):
                nc.tensor.matmul(
                    psv[:, 0:DO], xv[t][:, ki, tt * KT:(tt + 1) * KT],
                    wv_sb[:, ki, :], start=(ki == 0), stop=(ki == KI - 1))
            nc.vector.tensor_copy(
                vt[t][:, tt, :, 0:HD],
                psv[:, 0:DO].rearrange("p (h e) -> p h e", h=NH))
        return emit

    def unit_oproj(t, mtt, n, alt_copy=False):
        def emit():
            ps = psA.tile([128, QT], f32, tag="ps")
            for kc in range(DO // 128):
                nc.tensor.matmul(
                    ps[:], oTt[t][:, kc, mtt * KT:(mtt + 1) * KT],
                    wo_sb[:, kc, n * QT:(n + 1) * QT],
                    start=(kc == 0), stop=(kc == DO // 128 - 1))
            ob = obuf.tile([128, QT], bf16, tag="ob")
            if alt_copy:
                nc.scalar.copy(ob[:], ps[:])
            else:
                nc.vector.tensor_copy(ob[:], ps[:])
            mt = t * NT + mtt
            nc.sync.dma_start(
                out_d[mt * 128:(mt + 1) * 128, n * QT:(n + 1) * QT], ob[:])
        return emit

    def proj_units(t):
        return ([unit_proj_qk(t, d, w) for d in range(2) for w in ("q", "k")]
                + [unit_proj_v(t, tt) for tt in range(NT)])

    def oproj_units(t, alt=False):
        # alt: alternate the PSUM->SBUF copy between ACT and DVE so the
        # final (tail) out-projection is not paced by a single engine
        return [unit_oproj(t, mtt, n, alt_copy=alt and (mtt + n) % 2 == 0)
                for mtt in range(NT) for n in range(D // QT)]


    def attn_block(qt, fillers):
        """Attention for block qt with filler units interleaved into the PE
        stream (emitted just before each PV pair so a pending exp never
        head-of-line-blocks independent matmuls). Returns the deferred PE
        part of heads 2,3's normalization for the caller to emit later."""
        LEADP = _opts["leadp"]
        nkt = (qt + 1) * NT
        npairs = nkt // 2
        total_steps = NH * npairs
        fillers = deque(fillers)
        fill_acc, fill_step = 0.0, (len(fillers) / total_steps
                                    if total_steps else 0.0)
        # row-sum rows must sit at partition bases 0/32 (engine partition
        # offsets are 32-aligned and base 96 is illegal for PE):
        # heads 0,1 -> rsA rows 0,32; heads 2,3 -> rsB rows 0,32
        rsAB = [small.tile([64, QT], f32, tag="rsA", name="rsA"),
                small.tile([64, QT], f32, tag="rsB", name="rsB")]
        ouns = []

        def norm_dve(h):
            # DVE half of the normalization chain for heads 2h, 2h+1
            rcp = small.tile([64, QT], f32, tag="rcp")
            nc.vector.reciprocal_approx_fast(rcp[:], rsAB[h][:])
            rcpb = small.tile([64, QT], bf16, tag="rcpb", bufs=4)
            nc.vector.tensor_copy(rcpb[:], rcp[:])
            return rcpb

        def norm_pe_units(h, rcpb):
            # PE half (broadcast via ones-matmul) + final scale, as filler
            # units so they're emitted well after the DVE chain was queued
            def mk(j):
                def emit():
                    poff = (j % 2) * HD
                    psb = psA.tile([128, QT], f32, tag="ps")
                    nc.tensor.matmul(psb[0:HD, :],
                                     ones_bc[32 * (j % 2):32 * (j % 2) + 1, :],
                                     rcpb[32 * (j % 2):32 * (j % 2) + 1, :],
                                     start=True, stop=True)
                    bc = small.tile([HD, QT], bf16, tag="bc", bufs=4)
                    nc.vector.tensor_copy(bc[:], psb[0:HD, :])
                    nc.vector.tensor_mul(oTt[qt][poff:poff + HD, h, :],
                                         ouns[j][:], bc[:])
                return emit
            return [mk(2 * h), mk(2 * h + 1)]

        warmup(2)
        # one continuous score/exp/PV pipeline across ALL heads of the block:
        # head j+1's scores issue while head j's last PVs drain, so there is
        # no LEADP drain bubble at head boundaries
        chunks = [(j, p) for j in range(NH) for p in range(npairs)]
        psos = {}
        pending = deque()
        for idx in range(len(chunks) + LEADP):
            if idx < len(chunks):
                j, p = chunks[idx]
                poff = (j % 2) * HD
                d = j // 2
                if p == 0:
                    psos[j] = psO.tile([HD + 1, QT], f32, tag="pso",
                                       name="pso")
                qh = qTt[qt][poff:poff + HD, d, :]
                pss = psS.tile([128, 2, QT], f32, tag="pss")
                pt = ptp.tile([128, 2, QT], bf16, tag="pt")
                info = []
                for i in (0, 1):
                    kt = 2 * p + i
                    r = kt - qt * NT
                    co = max(r, 0) * KT
                    w = QT - co
                    kh = kTt[kt // NT][poff:poff + HD, d,
                                       (kt % NT) * KT:(kt % NT + 1) * KT]
                    nc.tensor.matmul(pss[:, i, 0:w], kh, qh[:, co:QT],
                                     start=True, stop=True)
                    info.append((kt, co, w, r))
                if _opts["pair_exp"]:
                    # one exp covers the pair, clipped to the wider chunk's
                    # width (pt beyond each chunk's own width is garbage and
                    # never read)
                    w0 = info[0][2]
                    nc.scalar.activation(pt[:, :, 0:w0], pss[:, :, 0:w0],
                                         Exp, scale=0.125)
                else:
                    for i, (kt, co, w, r) in enumerate(info):
                        nc.scalar.activation(pt[:, i, 0:w],
                                             pss[:, i, 0:w], Exp,
                                             scale=0.125)
                for i, (kt, co, w, r) in enumerate(info):
                    if r >= 0:
                        if _opts["mask_engine"] == "pool":
                            nc.gpsimd.tensor_mul(
                                pt[:, i, 0:KT], pt[:, i, 0:KT], tri_sb[:])
                        else:
                            nc.vector.tensor_mul(
                                pt[:, i, 0:KT], pt[:, i, 0:KT], tri_sb[:])
                pending.append((j, p, pt, info))
            # fillers go in front of the PV pair: PV waits on exp, the
            # filler must not sit behind it in the PE FIFO
            fill_acc += fill_step
            while fill_acc >= 1.0 and fillers:
                fillers.popleft()()
                fill_acc -= 1.0
            if idx >= LEADP:
                j, p, pt, info = pending.popleft()
                pso = psos[j]
                for i, (kt, co, w, r) in enumerate(info):
                    nc.tensor.matmul(
                        pso[:, co:QT], vt[kt // NT][:, kt % NT, j, :],
                        pt[:, i, 0:w],
                        start=(kt == 0), stop=(kt == nkt - 1))
                if p == npairs - 1:
                    # head complete: free the PSUM accumulator, stash the
                    # row-sum row at its 32-aligned slot
                    oun = small.tile([HD, QT], bf16, tag="oun", bufs=5)
                    nc.vector.tensor_copy(oun[:], pso[0:HD, :])
                    nc.vector.tensor_copy(
                        rsAB[j // 2][32 * (j % 2):32 * (j % 2) + 1, :],
                        pso[HD:HD + 1, :])
                    ouns.append(oun)
                    if j == 1:
                        # heads 0,1 complete (copies emitted): overlap their
                        # norm with heads 2,3's attention
                        fillers.extend(norm_pe_units(0, norm_dve(0)))
        while fillers:
            fillers.popleft()()
        return norm_pe_units(1, norm_dve(1))

    # ---- software-pipelined schedule ----
    # carry (the deferred norm-PE units) is placed mid-filler-stream so it
    # never reaches the PE before its DVE reciprocal chain has drained
    warmup(8)
    for u in proj_units(0):
        warmup(2)
        u()
    c = attn_block(0, proj_units(1))
    c = attn_block(1, proj_units(2) + c)
    c = attn_block(2, proj_units(3) + c + oproj_units(0))
    c = attn_block(3, oproj_units(1) + c + oproj_units(2))
    warmup(4)
    for u in c:
        u()
    for u in oproj_units(3, alt=True):
        u()


def _bf16(x: np.ndarray) -> np.ndarray:
    return np.ascontiguousarray(np.asarray(x, np.float32)).astype(
        ml_dtypes.bfloat16)


def _mask_tiles() -> np.ndarray:
    i = np.arange(128)[:, None]
    j = np.arange(KT)[None, :]
    return (j >= i).astype(np.float32)


def make_in_maps(query, key, value, Wq, Wk, Wv, Wo):
    query = np.asarray(query, np.float32)
    key = np.asarray(key, np.float32)
    value = np.asarray(value, np.float32)
    Wq = np.asarray(Wq, np.float32)
    Wk = np.asarray(Wk, np.float32)
    Wv = np.asarray(Wv, np.float32)
    Wo = np.asarray(Wo, np.float32)
    cm = _bf16(_mask_tiles())
    in_maps = []
    for c in range(NCORES):
        b, hg = divmod(c, NCORES // B)
        sl = slice(hg * DO, (hg + 1) * DO)
        in_maps.append({
            "xqT": _bf16(query[b].T),
            "xkT": _bf16(key[b].T),
            "xvT": _bf16(value[b].T),
            "wqT": _bf16(Wq[sl].T),
            "wkT": _bf16(Wk[sl].T),
            "wvT": _bf16(Wv[sl].T),
            "woT": _bf16(Wo[:, sl].T),
            "cmask": cm,
        })
    return in_maps


def kernel(query, key, value, freqs_complex_form, mask, Wq, Wk, Wv, Wo):
    if "nc" not in _cache:
        _cache["nc"] = _build()
    nc = _cache["nc"]
    in_maps = make_in_maps(query, key, value, Wq, Wk, Wv, Wo)
    res = run_bass_kernel_spmd(nc, in_maps, list(range(NCORES)))
    npg = NCORES // B
    parts = [np.asarray(res.results[c]["out"], dtype=np.float32)
             for c in range(NCORES)]
    return np.stack(
        [np.sum(parts[b * npg:(b + 1) * npg], axis=0) for b in range(B)]
    ).astype(np.float32)
